# revision 20
# baseline (speedup 1.0000x reference)
"""NT-Xent contrastive loss on 8 Trainium2 NeuronCores.

Math (reference): z = l2-normalize rows of concat(emb_i, emb_j) -> [8192, 512].
sim = (z @ z.T) / T with T = 0.5.  denom_r = sum_j exp(sim_rj) - exp(sim_rr),
sim_rr = 1/T exactly, so subtract e^2.  pos pair sim[k, k+N] = 2*cos_k.
loss = (sum_r log(denom_r) - 4 * sum_k cos_k) / 8192.

Sharding: data-parallel over rows of sim.  Each core computes a 1024-row
block of sim against all 8192 columns, reduces to one partial scalar, plus
a 512-pair slice of the positive-pair cosines.  Host sums the 8 partials.

Each core's repsT copy is host-ROLLED so its own 1024 columns sit at
[0:1024]: the DoubleRow lhsT is then a uniform slice of the group-0 fp8
tiles, eliminating a whole separate lhsT prep pass (row sums are
invariant to column order; the self-term stays exp(2)).

Device pipeline per core (identical SPMD program, per-core data):
  - inputs are host-cast to bf16; repsT [512, 8192] streamed in
    [128, 2048] tiles
  - column sums of squares via ones[128,128]-matmul of bf16 squares
    (DVE tensor_mul at 2x); PSUM result is replicated across partitions
  - B = exp(-0.5*ln(ss) + ln 16) = 16/sqrt(ss) on ACT (one table set)
  - zT = st * B cast straight to fp8e4 (values ~N(0, 0.7), well inside
    e4m3 range); tiles are laid out [128, 2, W] so each DoubleRow matmul
    contracts K=256 in one pass
  - main matmul: perf_mode=DoubleRow, lhsT = own 1024 normalized cols
    [128, 2, 1024] fp8, rhs = all 8192 cols, 2 DR chunks cover K=512,
    PSUM groups [128, 2048]
  - the PSUM gram is 256x the cosine (16x quant scale on each side), so
    ACT exp uses scale 2/256 with accum_out -> row sums
  - ln(denom - e^2), reduce, DMA 2 partial scalars; host combines
"""

import functools
import math

import numpy as np

import concourse.bacc as bacc
import concourse.bass as bass
import concourse.tile as tile
from concourse import mybir
from concourse.bass_utils import run_bass_kernel_spmd
from concourse.hw_specs import get_activation_tables as _orig_gat

F32 = mybir.dt.float32
BF16 = mybir.dt.bfloat16
FP8 = mybir.dt.float8e4
AF = mybir.ActivationFunctionType
ALU = mybir.AluOpType
DR = mybir.MatmulPerfMode.DoubleRow

N_CORES = 8
N = 4096              # rows per input
D = 512               # embedding dim
M = 2 * N             # 8192 rows of sim
ROWS_PER_CORE = M // N_CORES      # 1024
POS_PER_CORE = N // N_CORES       # 512
D_CH = D // 128       # 4 contraction chunks
E2 = float(math.exp(2.0))
INV_T = 2.0           # 1 / temperature
GW = 2048             # column-group width
QS = 16.0             # fp8 quantization scale per operand
LN_QS = float(math.log(QS))

_ONE_SET = "natural_log_exp_and_others"


@functools.cache
def _patched_gat(arch):
    """Pin every ACT function this kernel uses to one table set so the
    table-load chooser emits a single ACT_TABLE_LOAD (the default
    first-match policy alternates sets on every Ln<->Exp transition,
    costing ~2.7us per switch)."""
    t = dict(_orig_gat(arch))
    if _ONE_SET not in t:
        return t
    mine = {AF.Exp, AF.Ln, AF.Square, AF.Copy, AF.Identity}
    return {
        name: (s if name == _ONE_SET else (set(s) - mine))
        for name, s in t.items()
    }


def build_program():
    bacc.get_activation_tables = _patched_gat

    nc = bacc.Bacc(
        "TRN2",
        target_bir_lowering=False,
        debug=False,
        num_devices=N_CORES,
    )

    # repsT is rolled per core so the core's own 1024 columns sit at
    # [0:1024]; lhsT is then a uniform slice of the group-0 fp8 tiles
    # (row sums are invariant to column order).
    repsT = nc.dram_tensor("repsT", [D, M], BF16, kind="ExternalInput")
    pi = nc.dram_tensor("pi", [POS_PER_CORE, D], BF16, kind="ExternalInput")
    pj = nc.dram_tensor("pj", [POS_PER_CORE, D], BF16, kind="ExternalInput")
    out_d = nc.dram_tensor("out", [2, 1], F32, kind="ExternalOutput")

    with tile.TileContext(nc) as tc:
        import contextlib

        with contextlib.ExitStack() as ctx:
            const = ctx.enter_context(tc.tile_pool(name="const", bufs=1))
            big = ctx.enter_context(tc.tile_pool(name="big", bufs=1))
            stage = ctx.enter_context(tc.tile_pool(name="stage", bufs=9))
            sqp = ctx.enter_context(tc.tile_pool(name="sqp", bufs=3))
            bpool = ctx.enter_context(tc.tile_pool(name="bpool", bufs=3))
            lnp = ctx.enter_context(tc.tile_pool(name="lnp", bufs=2))
            ztp = ctx.enter_context(tc.tile_pool(name="ztp", bufs=3))
            posp = ctx.enter_context(tc.tile_pool(name="posp", bufs=4))
            sink = ctx.enter_context(tc.tile_pool(name="sink", bufs=2))
            esink = ctx.enter_context(tc.tile_pool(name="esink", bufs=2))

            ones128 = const.tile([128, 128], BF16)
            nc.vector.memset(ones128[:], 1.0)
            ones_f = const.tile([128, 1], F32)
            nc.vector.memset(ones_f[:], 1.0)
            neg_e2 = const.tile([128, 1], F32)
            nc.vector.memset(neg_e2[:], -E2)
            ln_qs = const.tile([128, 1], F32)
            nc.vector.memset(ln_qs[:], LN_QS)

            dacc = big.tile([128, 32], F32, tag="dacc")
            pos_ssi = big.tile([128, 4], F32, tag="pos_ssi")
            pos_ssj = big.tile([128, 4], F32, tag="pos_ssj")
            pos_dot = big.tile([128, 4], F32, tag="pos_dot")

            pp_main = ctx.enter_context(
                tc.tile_pool(name="pp_main", bufs=2, space="PSUM")
            )

            def emit_prep_group(src, col0, w, dst, label, act_squares=0):
                """Normalize w columns of src starting at col0 into dst
                (2 DoubleRow tiles [128, 2, w] fp8).  w in {1024, 2048}.
                act_squares: how many of the 4 chunk squares run on the
                (early-idle) ACT engine instead of the DVE."""
                nk = w // 512
                pt = pp_main.tile([128, GW], F32, tag="pp_main",
                                  name=f"ssg_{label}")
                sts = []
                for d in range(D_CH):
                    st = stage.tile([128, GW], BF16, tag="stage",
                                    name=f"st_{label}_{d}")
                    nc.sync.dma_start(
                        st[0:128, 0:w], src[bass.ts(d, 128), col0 : col0 + w]
                    )
                    sts.append(st)
                    sqt = sqp.tile([128, GW], BF16, tag="sqp",
                                   name=f"sq_{label}_{d}")
                    if d < act_squares:
                        nc.scalar.activation(sqt[0:128, 0:w],
                                             st[0:128, 0:w], AF.Square)
                    else:
                        nc.vector.tensor_mul(sqt[0:128, 0:w],
                                             st[0:128, 0:w], st[0:128, 0:w])
                    for k in range(nk):
                        nc.tensor.matmul(
                            pt[:, bass.ts(k, 512)],
                            ones128[:], sqt[:, bass.ts(k, 512)],
                            start=(d == 0), stop=(d == D_CH - 1),
                        )
                bt = bpool.tile([128, GW], BF16, tag="bpool",
                                name=f"B_{label}")
                lt = lnp.tile([128, GW], F32, tag="lnp")
                nc.scalar.activation(lt[0:128, 0:w], pt[0:128, 0:w], AF.Ln)
                nc.scalar.activation(bt[0:128, 0:w], lt[0:128, 0:w],
                                     AF.Exp, scale=-0.5, bias=ln_qs[:])
                for d in range(D_CH):
                    nc.vector.tensor_mul(
                        dst[d // 2][0:128, d % 2 : d % 2 + 1, 0:w],
                        sts[d][0:128, 0:w],
                        bt[0:128, 0:w],
                    )

            def new_zgroup(jg):
                if jg == 0:
                    # group 0 doubles as lhsT for every mains group; keep
                    # it in the persistent pool, outside the ztp rotation
                    return [big.tile([128, 2, GW], FP8, tag=f"z0_{p}",
                                     name=f"zt_0_{p}") for p in range(2)]
                return [ztp.tile([128, 2, GW], FP8, tag=f"zt{p}",
                                 name=f"zt_{jg}_{p}") for p in range(2)]

            def emit_mains(jg, zg, lhsT):
                for i in range(8):
                    pt = pp_main.tile([128, GW], F32, tag="pp_main",
                                      name=f"mm_{jg}_{i}")
                    for p in range(2):
                        for jj in range(4):
                            nc.tensor.matmul(
                                pt[:, bass.ts(jj, 512)],
                                lhsT[p][:, :, bass.ts(i, 128)],
                                zg[p][:, :, bass.ts(jj, 512)],
                                start=(p == 0), stop=(p == 1),
                                perf_mode=DR,
                            )
                    es = esink.tile([128, GW], BF16, tag="esink")
                    k = i * 4 + jg
                    nc.scalar.activation(
                        es[:], pt[:], AF.Exp, scale=INV_T / (QS * QS),
                        accum_out=dacc[:, k : k + 1],
                    )

            def emit_pos():
                for t in range(4):
                    pit = posp.tile([128, D], BF16, tag="posp")
                    nc.sync.dma_start(pit[:], pi[bass.ts(t, 128), :])
                    pjt = posp.tile([128, D], BF16, tag="posp")
                    nc.sync.dma_start(pjt[:], pj[bass.ts(t, 128), :])
                    for src0, src1, acc in (
                        (pit, pit, pos_ssi),
                        (pjt, pjt, pos_ssj),
                        (pit, pjt, pos_dot),
                    ):
                        snk = sink.tile([128, D], BF16, tag="sink")
                        nc.vector.tensor_mul(snk[:], src0[:], src1[:])
                        nc.vector.tensor_reduce(
                            acc[:, t : t + 1], snk[:],
                            axis=mybir.AxisListType.X, op=ALU.add,
                        )
                lssi = big.tile([128, 4], F32, tag="lssi")
                lssj = big.tile([128, 4], F32, tag="lssj")
                nc.scalar.activation(lssi[:], pos_ssi[:], AF.Ln)
                nc.scalar.activation(lssj[:], pos_ssj[:], AF.Ln)
                lsum = big.tile([128, 4], F32, tag="lsum")
                nc.vector.tensor_add(lsum[:], lssi[:], lssj[:])
                rinv_ij = big.tile([128, 4], F32, tag="rinv_ij")
                nc.scalar.activation(rinv_ij[:], lsum[:], AF.Exp, scale=-0.5)
                posk = big.tile([128, 4], F32, tag="posk")
                nc.vector.tensor_mul(posk[:], pos_dot[:], rinv_ij[:])
                return posk

            # ------- software-pipelined schedule ----------------------------
            zg = {}
            zg[0] = new_zgroup(0)
            emit_prep_group(repsT, 0, GW, zg[0], "g0")
            zg[1] = new_zgroup(1)
            emit_prep_group(repsT, GW, GW, zg[1], "g1")
            emit_mains(0, zg[0], zg[0])
            zg[2] = new_zgroup(2)
            emit_prep_group(repsT, 2 * GW, GW, zg[2], "g2")
            emit_mains(1, zg[1], zg[0])
            zg[3] = new_zgroup(3)
            emit_prep_group(repsT, 3 * GW, GW, zg[3], "g3")
            posk = emit_pos()
            emit_mains(2, zg[2], zg[0])
            emit_mains(3, zg[3], zg[0])

            # ------- final reduction ----------------------------------------
            dn = big.tile([128, 8], F32, tag="dn")
            nc.vector.tensor_reduce(
                dn[:], dacc[:].rearrange("p (i g) -> p i g", g=4),
                axis=mybir.AxisListType.X, op=ALU.add,
            )
            ld = big.tile([128, 8], F32, tag="ld")
            nc.scalar.activation(ld[:], dn[:], AF.Ln, bias=neg_e2[:])
            fin = big.tile([128, 2], F32, tag="fin")
            nc.vector.tensor_reduce(
                fin[:, 0:1], ld[:], axis=mybir.AxisListType.X, op=ALU.add
            )
            nc.vector.tensor_reduce(
                fin[:, 1:2], posk[:], axis=mybir.AxisListType.X, op=ALU.add
            )
            fmm = pp_main.tile([128, GW], F32, tag="pp_main", name="fmm")
            nc.tensor.matmul(fmm[0:2, 0:1], fin[:], ones_f[:], start=True,
                             stop=True)
            outsb = big.tile([2, 1], F32, tag="outsb")
            nc.vector.tensor_copy(outsb[:], fmm[0:2, 0:1])
            nc.sync.dma_start(out_d[:], outsb[:])

    nc.compile()
    return nc


_NC_CACHE = None


def _get_program():
    global _NC_CACHE
    if _NC_CACHE is None:
        _NC_CACHE = build_program()
    return _NC_CACHE


def make_in_maps(emb_i: np.ndarray, emb_j: np.ndarray):
    import ml_dtypes

    bf16 = ml_dtypes.bfloat16
    emb_i = np.asarray(emb_i, dtype=np.float32)
    emb_j = np.asarray(emb_j, dtype=np.float32)
    reps = np.concatenate([emb_i, emb_j], axis=0)          # [8192, 512]
    repsT = np.ascontiguousarray(reps.T).astype(bf16)      # [512, 8192]
    in_maps = []
    for c in range(N_CORES):
        in_maps.append(
            {
                # roll so this core's own 1024 columns sit at [0:1024];
                # row sums don't depend on column order
                "repsT": np.ascontiguousarray(
                    np.roll(repsT, -c * ROWS_PER_CORE, axis=1)
                ),
                "pi": emb_i[c * POS_PER_CORE : (c + 1) * POS_PER_CORE]
                .astype(bf16),
                "pj": emb_j[c * POS_PER_CORE : (c + 1) * POS_PER_CORE]
                .astype(bf16),
            }
        )
    return in_maps


def combine_outputs(results):
    ld_sum = 0.0
    cos_sum = 0.0
    for r in results:
        o = np.asarray(r["out"], dtype=np.float64).reshape(-1)
        ld_sum += o[0]
        cos_sum += o[1]
    loss = (ld_sum - 2.0 * INV_T * cos_sum) / float(M)
    return np.float32(loss)


def kernel(emb_i: np.ndarray, emb_j: np.ndarray) -> np.ndarray:
    nc = _get_program()
    in_maps = make_in_maps(emb_i, emb_j)
    res = run_bass_kernel_spmd(nc, in_maps, list(range(N_CORES)))
    return combine_outputs(res.results)
